# revision 29
# baseline (speedup 1.0000x reference)
"""Bass/Trainium2 kernel for nn_CustomBBoxLoss.

Reference computation:
    A1 = pred.sum(axis=(0,1));  A2 = (pred**2).sum(axis=(0,1))      # [H, W]
    s1[b] = sum of A1 over box b's region;  s2[b] likewise for A2
    per_box = (s2 - 2*cls*s1 + cls^2*cnt) / cnt;  loss = per_box.mean()

Each region sum is a bilinear form  s[b] = rowmask_b^T @ A @ colmask_b
with 0/1 interval masks.  The row-mask contraction (fused with the
(B,C)-map reduction) runs on the PE as matmuls with the transposed row
masks as stationary operands; the col-mask contraction is a fused
multiply-reduce on the vector engine over the closed PSUM groups.

Sharding: pred's [2048, 2048] spatial plane is split into a 4x2 grid
(512 rows x 1024 cols per core) over 8 cores.  The interval masks are
precomputed host-side from the box index vectors (per core, shifted to
the core's origin) and staged as small bf16 inputs — index-metadata
preparation, so no engine cycles are spent building masks on device.
Each core emits per-box partial sums; the host sums the 8 partials
(the "all-reduce") and applies the closed-form per-box formula.

Pipeline structure (per core):
  - 14 streaming DMAs fill all-resident SBUF stage tiles back-to-back:
    the SDMA engines run gap-free at the ~360 GB/s HBM roofline.
  - Per stage (one map-pair x 1024 cols): DVE pair-add -> bf16 t1;
    ACT squares -> bf16 sq0/sq1; PE accumulates rowmask^T @ {t1,sq}
    into 8 PSUM groups (box-half x quantity x col-chunk).
  - The three map-pair sums of each row tile are folded on DVE (bf16,
    2x rate) so each row tile's t1 streams through the PE once.
  - The final stage is split into two 512-col half-stages so the n=0
    PSUM groups close early and their col-mask applies overlap the last
    DMA; each half-stage orders t1 before squares so applies pipeline
    behind the closing matmuls.
"""

import numpy as np

H = W = 2048
B, C, N = 2, 3, 256
MAPS = B * C                      # 6
RB, CB = 4, 2                     # row-blocks x col-blocks = 8 cores
ROWS, COLS = H // RB, W // CB     # 512 x 1024 per core
P = 128                           # partitions
NRT = ROWS // P                   # 4 row tiles per core
NPAIR = MAPS // 2                 # 3 map pairs
NK = NRT * NPAIR                  # 12 logical stages per core
NCH = COLS // 512                 # 2 col chunks of 512
NB = N // P                       # 2 box halves

_CACHE = {}


def _build_module():
    import concourse.bacc as bacc
    import concourse.mybir as mybir
    import concourse.tile as tile

    f32 = mybir.dt.float32
    bf16 = mybir.dt.bfloat16
    Alu = mybir.AluOpType

    nc = bacc.Bacc("TRN2", target_bir_lowering=False, debug=False)

    # stages 0..10 full [P, 2048]; stage 11 = two halves [P, 1024] packed
    # as [m0c0|m1c0] and [m0c1|m1c1] at the tail of pred_part.
    pred_part = nc.declare_dram_parameter("pred_part", [NK, P, 2048], f32, isOutput=False)
    # host-built interval masks: rm[p, rt*256 + box], cm[p, b*1024 + col]
    rm_in = nc.declare_dram_parameter("rm", [P, NRT * N], bf16, isOutput=False)
    cm_in = nc.declare_dram_parameter("cm", [P, NB * COLS], bf16, isOutput=False)
    # columns: b*4 + q*2 + n -> partial region sums for boxes
    # [b*128, (b+1)*128), quantity q, col chunk n (host sums the chunks)
    out_s = nc.declare_dram_parameter("out_s", [P, 8], f32, isOutput=True)

    with tile.TileContext(nc) as tc:
        with (
            tc.tile_pool(name="persist", bufs=1) as pp,
            tc.tile_pool(name="stage", bufs=11) as stage_pool,
            tc.tile_pool(name="halfst", bufs=4) as half_pool,
            tc.tile_pool(name="t1p", bufs=4) as t1_pool,
            tc.tile_pool(name="q01p", bufs=3) as q01_pool,
            tc.tile_pool(name="sqp", bufs=6) as sq_pool,
            tc.tile_pool(name="scratch", bufs=4) as scr_pool,
            tc.tile_pool(name="psum", bufs=1, space="PSUM") as psum_pool,
        ):
            # ---- mask DMAs ride the scalar HWDGE queue so the sync queue
            # starts streaming pred immediately ----
            rm_all = pp.tile([P, NRT * N], bf16, tag="rm_all", name="rm_all")
            cm_all = pp.tile([P, NB * COLS], bf16, tag="cm_all", name="cm_all")
            nc.scalar.dma_start(rm_all[:], rm_in.ap()[:])
            nc.scalar.dma_start(cm_all[:], cm_in.ap()[:])

            def rmaskT(rt, b):
                return rm_all[:, rt * N + b * P: rt * N + (b + 1) * P]

            def cmask(b):
                return cm_all[:, b * COLS:(b + 1) * COLS]

            # ---- big streaming DMAs: all-resident stages, issued back to
            # back with no reuse waits; first stage split across halves to
            # cut pipeline-fill latency ----
            stages = []
            for k in range(NK - 1):
                stage = stage_pool.tile([P, 2048], f32, tag="stage", name="stage")
                if k == 0:
                    nc.sync.dma_start(stage[:, :COLS], pred_part.ap()[k][:, :COLS])
                    nc.sync.dma_start(stage[:, COLS:], pred_part.ap()[k][:, COLS:])
                else:
                    nc.sync.dma_start(stage[:], pred_part.ap()[k])
                stages.append(stage)
            quarters = []
            for h in range(4):
                hst = half_pool.tile([P, 512], f32, tag="hst", name="hst")
                nc.sync.dma_start(hst[:], pred_part.ap()[NK - 1][:, h * 512:(h + 1) * 512])
                quarters.append(hst)

            # ---- PSUM groups: (box half, quantity, col chunk) ----
            ps = {}
            for b in range(NB):
                for q in range(2):
                    for n in range(NCH):
                        ps[(b, q, n)] = psum_pool.tile(
                            [P, 512], f32, tag=f"ps{b}{q}{n}", name=f"ps{b}{q}{n}")

            # ---- PE prewarm while DMAs fill (HAM clock gate) ----
            junk = pp.tile([P, 512], bf16, tag="junk", name="junk")
            nc.gpsimd.memset(junk[:], 0.0)
            for w in range(8):
                nc.tensor.matmul(ps[(0, 0, 0)][:], junk[:, :P], junk[:],
                                 start=True, stop=True)

            # stages whose two squares are pre-summed on DVE (halves the PE
            # streaming for them); measured: DVE adds are ~1.1-3ns/elem so
            # this trade is a net loss — keep empty
            FOLD_SQ = set()
            # stages whose second square runs on GpSimd (measured ~2.9us per
            # [128,1024] op — too slow to help): keep empty
            GPS_SQ1 = set()

            # matmul bookkeeping: start on first contribution per group; stop
            # flags are set explicitly by the final quarter-stage matmuls
            started = {}

            def mm(group, lhsT, rhs, kind, last=False, region=None):
                first = group not in started
                started[group] = True
                out = ps[group][:] if region is None else \
                    ps[group][:, region[0]:region[1]]
                nc.tensor.matmul(out, lhsT, rhs, start=first, stop=last)
                return last

            # ---- apply helpers ----
            # s_all columns: b*4 + q*2 + n; each written by exactly one op.
            s_all = pp.tile([P, 8], f32, tag="s_all", name="s_all")

            applied = set()

            def apply_group(b, q, n):
                g = b * 4 + q * 2 + n
                scr = scr_pool.tile([P, 512], f32, tag="scr", name="scr")
                nc.vector.scalar_tensor_tensor(
                    out=scr[:],
                    in0=ps[(b, q, n)][:],
                    scalar=1.0,
                    in1=cmask(b)[:, n * 512:(n + 1) * 512],
                    op0=Alu.mult,
                    op1=Alu.mult,
                    accum_out=s_all[:, g:g + 1],
                )
                applied.add((b, q, n))

            # ---- stream: pair-add + fold (DVE) + squares (ACT) + matmuls ----
            pair_tiles = {}
            for rt in range(NRT):
                for j in range(NPAIR):
                    k = rt * NPAIR + j
                    if k == NK - 1:
                        break                     # handled as half-stages below
                    stage = stages[k]

                    t1t = t1_pool.tile([P, COLS], bf16, tag="t1t", name="t1t")
                    nc.vector.tensor_add(t1t[:], stage[:, :COLS], stage[:, COLS:])
                    pair_tiles[(rt, j)] = t1t
                    sq0 = sq_pool.tile([P, COLS], bf16, tag="sq0", name="sq0")
                    sq1 = sq_pool.tile([P, COLS], bf16, tag="sq1", name="sq1")
                    nc.scalar.square(sq0[:], stage[:, :COLS])
                    if k in GPS_SQ1:
                        nc.gpsimd.tensor_mul(sq1[:], stage[:, COLS:], stage[:, COLS:])
                    else:
                        nc.scalar.square(sq1[:], stage[:, COLS:])

                    # squares stream grouped by box half; FOLD_SQ stages
                    # pre-sum the two squares on DVE (bf16, 2x rate) so they
                    # stream through the PE once
                    if k in FOLD_SQ:
                        ssum = sq_pool.tile([P, COLS], bf16, tag="ssum", name="ssum")
                        nc.vector.tensor_add(ssum[:], sq0[:], sq1[:])
                        for b in range(NB):
                            lhsT = rmaskT(rt, b)
                            for n in range(NCH):
                                mm((b, 1, n), lhsT, ssum[:, n * 512:(n + 1) * 512], "sq")
                    else:
                        for b in range(NB):
                            lhsT = rmaskT(rt, b)
                            for si, sqt in ((0, sq0), (1, sq1)):
                                for n in range(NCH):
                                    mm((b, 1, n), lhsT, sqt[:, n * 512:(n + 1) * 512], "sq")

                    # t1: map pairs folded on GpSimd so each row tile's sum
                    # streams through the PE once (rt 0-2: all 6 maps; rt 3:
                    # pairs 0,1 — pair 2 is the split final stage)
                    if j == 1:
                        q01 = q01_pool.tile([P, COLS], bf16, tag="q01", name="q01")
                        nc.vector.tensor_add(q01[:], pair_tiles[(rt, 0)][:], t1t[:])
                        pair_tiles[(rt, "q01")] = q01
                        if rt == NRT - 1:
                            for b in range(NB):
                                lhsT = rmaskT(rt, b)
                                for n in range(NCH):
                                    mm((b, 0, n), lhsT, q01[:, n * 512:(n + 1) * 512], "t1")
                    elif j == 2:
                        q012 = q01_pool.tile([P, COLS], bf16, tag="q01", name="q01")
                        nc.vector.tensor_add(q012[:], pair_tiles[(rt, "q01")][:], t1t[:])
                        for b in range(NB):
                            lhsT = rmaskT(rt, b)
                            for n in range(NCH):
                                mm((b, 0, n), lhsT, q012[:, n * 512:(n + 1) * 512], "t1")

            # ---- final stage as four 256-col quarter-stages (rt=3, j=2) so
            # the tail chain after the last DMA byte is minimal and the n=0
            # applies overlap the n=1 quarters ----
            rt = NRT - 1
            for nq in range(4):
                n, sub = nq // 2, nq % 2
                region = (sub * 256, (sub + 1) * 256)
                hst = quarters[nq]
                t1h = t1_pool.tile([P, 256], bf16, tag="t1h", name="t1h")
                nc.vector.tensor_add(t1h[:], hst[:, :256], hst[:, 256:])
                sqh0 = sq_pool.tile([P, 256], bf16, tag="sqh0", name="sqh0")
                sqh1 = sq_pool.tile([P, 256], bf16, tag="sqh1", name="sqh1")
                nc.scalar.square(sqh0[:], hst[:, :256])
                nc.scalar.square(sqh1[:], hst[:, 256:])
                closing = (sub == 1)
                for b in range(NB):
                    lhsT = rmaskT(rt, b)
                    mm((b, 0, n), lhsT, t1h[:], "t1", last=closing, region=region)
                    if closing:
                        apply_group(b, 0, n)
                    mm((b, 1, n), lhsT, sqh0[:], "sq", region=region)
                    mm((b, 1, n), lhsT, sqh1[:], "sq", last=closing, region=region)
                    if closing:
                        apply_group(b, 1, n)

            # sanity: every group must be closed and applied
            assert len(applied) == NB * 2 * NCH

            # ---- out DMA on the idle sync queue; host merges col chunks ----
            nc.sync.dma_start(out_s.ap()[:], s_all[:])

    nc.compile()
    return nc


def _get_module():
    if "nc" not in _CACHE:
        _CACHE["nc"] = _build_module()
    return _CACHE["nc"]


def _make_in_maps(pred, box_y, box_x, box_h, box_w):
    import ml_dtypes

    pred6 = np.ascontiguousarray(pred).reshape(MAPS, H, W)
    y = box_y.astype(np.int64)
    x = box_x.astype(np.int64)
    h = box_h.astype(np.int64)
    w = box_w.astype(np.int64)
    in_maps = []
    for core in range(RB * CB):
        rb, cb = divmod(core, CB)
        slab = pred6[:, rb * ROWS:(rb + 1) * ROWS, cb * COLS:(cb + 1) * COLS]
        a = slab.reshape(NPAIR, 2, NRT, P, COLS)       # [j, t, rt, p, c]
        a = np.ascontiguousarray(a.transpose(2, 0, 3, 1, 4))  # [rt, j, p, t, c]
        a = a.reshape(NK, P, 2048)
        # final stage: repack [m0(1024)|m1(1024)] into four quarter-stages
        # [m0c(256)|m1c(256)] each
        last = a[NK - 1].reshape(P, 2, 4, 256)          # [p, t, nq, c]
        a[NK - 1] = np.ascontiguousarray(
            last.transpose(0, 2, 1, 3)).reshape(P, 2048)  # [p, nq, t, c]

        # host-built interval masks (exact 0/1 in bf16)
        yl = y - rb * ROWS
        xl = x - cb * COLS
        rows = np.arange(ROWS)[:, None]                 # [512, 1]
        rmask = ((rows >= yl[None, :]) & (rows < (yl + h)[None, :]))  # [512, 256]
        rmask = rmask.reshape(NRT, P, N).transpose(1, 0, 2).reshape(P, NRT * N)
        cols = np.arange(COLS)[None, :]                 # [1, 1024]
        cmask = ((cols >= xl[:, None]) & (cols < (xl + w)[:, None]))  # [256, 1024]
        cmask = cmask.reshape(NB, P, COLS).transpose(1, 0, 2).reshape(P, NB * COLS)

        in_maps.append({
            "pred_part": a,
            "rm": rmask.astype(ml_dtypes.bfloat16),
            "cm": cmask.astype(ml_dtypes.bfloat16),
        })
    return in_maps


def _finalize(results, box_h, box_w, box_cls):
    s1 = np.zeros(N, np.float64)
    s2 = np.zeros(N, np.float64)
    for r in results:
        o = r["out_s"].astype(np.float64)  # [128, b*4 + q*2 + n]
        for b in range(NB):
            s1[b * P:(b + 1) * P] += o[:, b * 4 + 0] + o[:, b * 4 + 1]
            s2[b * P:(b + 1) * P] += o[:, b * 4 + 2] + o[:, b * 4 + 3]
    cnt = float(MAPS) * (box_h.astype(np.float64) * box_w.astype(np.float64))
    cls = box_cls.astype(np.float64)
    per_box = (s2 - 2.0 * cls * s1 + cls * cls * cnt) / cnt
    return np.asarray(per_box.mean(), dtype=np.float32)


def kernel(pred, box_y, box_x, box_h, box_w, box_cls, _bench=None):
    from concourse.bass_utils import run_bass_kernel_spmd

    pred = np.asarray(pred, dtype=np.float32)
    box_y = np.asarray(box_y, dtype=np.int32)
    box_x = np.asarray(box_x, dtype=np.int32)
    box_h = np.asarray(box_h, dtype=np.int32)
    box_w = np.asarray(box_w, dtype=np.int32)
    box_cls = np.asarray(box_cls, dtype=np.int32)

    nc = _get_module()
    in_maps = _make_in_maps(pred, box_y, box_x, box_h, box_w)
    kw = dict(_bench) if _bench else {}
    try:
        res = run_bass_kernel_spmd(nc, in_maps, core_ids=list(range(RB * CB)), **kw)
    except Exception:
        # transient NRT/device hiccups happen; one clean retry
        res = run_bass_kernel_spmd(nc, in_maps, core_ids=list(range(RB * CB)), **kw)
    if _bench is not None:
        _CACHE["last_results"] = res
    return _finalize(res.results, box_h, box_w, box_cls)
